# revision 100
# baseline (speedup 1.0000x reference)
"""Trainium2 Bass kernel for nn_Attention_66949950210549.

Dense transformer attention block:
  qkv = x @ qkv_w.T ; per-head LN on q,k ; RoPE (positions restart at N/2) ;
  softmax(q k^T * HD^-0.5 + cross-block log(0.5) bias) @ v ; proj.

Sharding: 8 cores = 2 (batch) x 4 (head groups of 4 heads).  Each core
computes its batch's qkv for its 4 heads, attention, and a partial
projection (row-parallel over the head channels); the host sums the 4
partials per batch in f32 (partials stored f16).

v5 design notes (cost-model driven; 235.8us -> 187.3us):
  - matmul cost on PE is proportional to the OUTPUT free size only, so AV
    runs transposed: out[q=128, d0..d63, denom] = e^T v per (head, q-tile,
    k-chunk) costs 65 cycles/chunk instead of 512 in the (d, q) layout.
    The softmax denominator accumulates in column 64 via a ones-column on
    v; normalization is a per-partition reciprocal+scale (no partition
    broadcast, no DRAM bounce).  o is PE-transposed per q-tile into (d, q)
    layout for the projection.
  - ONE set of PSUM pools is shared by all phases (sp tiles reuse the qkv
    qp slots, AV accumulators reuse the vp slots, proj/transposes reuse
    the tp slots), so there is no pool barrier at the phase boundary; a
    4th score tile per kc borrows the tp slots to widen the sp recycle
    loop (PE otherwise rate-locks to exp completions).
  - closed AV accumulators are EVACUATED to SBUF in three wide copies at
    pair end: the PSUM slots free immediately (short AV lag, no WAR
    coupling to the next pair) and the normalize multiplies move to the
    otherwise-idle Pool engine (SBUF-only access).
  - the last 4 qT/kT transposes are interleaved into the first attention
    pair's kc loop; deferred normalizes of each pair spread 1-per-kc into
    the next pair.  Per-tile v eviction is emitted right after the Square
    so the vp slot rotation stays off the LN chain's critical path.
  - act tables: a dummy Sqrt at t=0 pins the sqrt set under the initial
    DMA shadow; a dummy Exp after the last LN Sqrt switches sets during
    the rope tail so the first real softmax exp is never table-stalled.
  - exp is engine-split per kc (ACT 2.25 halves native Exp, DVE 1.75
    halves Schraudolph bit-trick i16 = rint(x*2^10/ln2 + (15*2^10-44.5)),
    bits as fp16); proj output copies split ACT/DVE.
  - all inputs are partition-major so each is one or two large contiguous
    DMAs (the DMA queues pay ~0.7us dispatch per transfer and the DMA
    device is FIFO across queues), streamed on one strictly-ordered
    queue in consumption order.
"""

import math
import os
import sys

sys.path.insert(0, "/opt/trn_rl_repo")

PHASES = os.environ.get("BASS_PHASES", "DEF")

import numpy as np

import concourse.bacc as bacc
import concourse.bass as bass
import concourse.tile as tile
from concourse import bass_utils, mybir

B, N, C = 2, 2048, 1024
H, HD = 16, 64
NCORES = 8
GH = 4  # head-group count (cores per batch)
NH = H // GH  # heads per core = 4
NG = 2 * NH  # LN groups per core (q heads + k heads) = 8
J = 3 * NH * HD  # qkv rows per core = 768
JA = J + NG  # + 8 mean columns = 776
NIN = N // 2  # rope positions restart here
NT = N // 128  # 16 row tiles
CCH = C // 128  # 8 contraction chunks
RB = NIN // 128  # rope table row blocks = 8
LOG_COND = math.log(0.5)
EPS = 1e-5
SCALE = HD ** -0.5  # 0.125

# Schraudolph fp16 exp: i16 = rint(x * SCH_S + SCH_C); bits as fp16.
SCH_S = float(2 ** 10) / math.log(2.0)
SCH_C = 15.0 * 2 ** 10 - 44.5

F32 = mybir.dt.float32
F16 = mybir.dt.float16
I16 = mybir.dt.int16
AF = mybir.ActivationFunctionType
AX = mybir.AxisListType
ALU = mybir.AluOpType

PAIRS = [(0, 0), (0, 1), (1, 0), (1, 1)]  # (nqh, hp)


def _bcast(ap, n, axis_insert=1):
    """Insert a 0-stride dim of size n into an AP (free-dim broadcast)."""
    new = list(ap.ap)
    new.insert(axis_insert, [0, n])
    return bass.AP(tensor=ap.tensor, offset=ap.offset, ap=new)


def build_nc(with_qb: bool, with_kb: bool, shared_t: bool = False, with_pb: bool = True):
    nc = bacc.Bacc("TRN2", target_bir_lowering=False, debug=False)

    # all inputs partition-major so each is one or two large contiguous
    # DMAs (the DMA queues pay ~0.7-1.3us dispatch latency per transfer)
    xt = nc.dram_tensor("xt", [128, CCH, N], F16, kind="ExternalInput")
    wt = nc.dram_tensor("wt", [128, CCH, JA], F16, kind="ExternalInput")
    pwt = nc.dram_tensor("pwt", [128, 2, C], F16, kind="ExternalInput")
    ident = nc.dram_tensor("ident", [128, 128], F16, kind="ExternalInput")
    nkinds_q = 6 if with_qb else 4
    nkinds_k = 6 if with_kb else 4
    tq = nc.dram_tensor("tq", [128, RB, nkinds_q, 32], F16, kind="ExternalInput")
    tk = None
    if not shared_t:
        tk = nc.dram_tensor("tk", [128, RB, nkinds_k, 32], F16, kind="ExternalInput")
    pb = None
    if with_pb:
        pb = nc.dram_tensor("pb", [C], F32, kind="ExternalInput")
    out_p = nc.dram_tensor("out_p", [N, C], F16, kind="ExternalOutput")
    dbg = None
    if os.environ.get("BASS_DEBUG"):
        dbg = {
            "dbg_qT": nc.dram_tensor("dbg_qT", [128, 2, N], F16, kind="ExternalOutput"),
            "dbg_kT": nc.dram_tensor("dbg_kT", [128, 2, N], F16, kind="ExternalOutput"),
            "dbg_oT": nc.dram_tensor("dbg_oT", [128, 2, N], F16, kind="ExternalOutput"),
            "dbg_v": nc.dram_tensor(
                "dbg_v", [128, NT, NH, HD + 1], F16, kind="ExternalOutput"
            ),
        }

    def rng(n, ph):
        return range(n if ph in PHASES else 0)

    with tile.TileContext(nc) as tc:
        with (
            tc.tile_pool(name="persist", bufs=1) as persist,
            tc.tile_pool(name="qkvps", bufs=3, space="PSUM") as qkvps,
            tc.tile_pool(name="tpps", bufs=2, space="PSUM") as tpps,
            tc.tile_pool(name="dwork", bufs=8) as dwork,
            tc.tile_pool(name="epool", bufs=32) as epool,
            tc.tile_pool(name="nwork", bufs=8) as nwork,
            tc.tile_pool(name="fwork", bufs=8) as fwork,
        ):
            # ---- persistent SBUF tensors --------------------------------
            xT_sb = persist.tile([128, CCH, N], F16)
            wT_sb = persist.tile([128, CCH, JA], F16)
            pwT_sb = persist.tile([128, 2, C], F16)
            v_sb = persist.tile([128, NT, NH, HD + 1], F16)
            qT_sb = persist.tile([128, 2, N], F16)
            kT_sb = persist.tile([128, 2, N], F16)
            oT_sb = persist.tile([128, 2, N], F16)
            tq_sb = persist.tile([128, RB, nkinds_q, 32], F16)
            id_sb = persist.tile([128, 128], F16)
            tk_sb = None
            if not shared_t:
                tk_sb = persist.tile([128, RB, nkinds_k, 32], F16)
            pb_rep = None

            cst = persist.tile([128, 4], F32)
            nc.vector.memset(cst[:, 0:1], EPS)
            nc.vector.memset(cst[:, 1:2], 0.0)
            nc.vector.memset(cst[:, 2:3], LOG_COND)
            nc.const_aps.aps[(F32, EPS)] = cst[:, 0:1]
            nc.const_aps.aps[(F32, 0.0)] = cst[:, 1:2]
            nc.const_aps.aps[(F32, LOG_COND)] = cst[:, 2:3]
            # Pin the sqrt act-table at t=0 (under the DMA shadow) so the
            # per-tile LN Sqrt never stalls on a table load.
            nc.scalar.activation(out=cst[:, 3:4], in_=cst[:, 0:1], func=AF.Sqrt)

            # ---- input DMAs in strict consumption order -----------------
            # sync queue carries the critical stream: half the weights, the
            # leading x columns, the rest of the weights, then x column
            # blocks in tile order, proj weights last.
            # single strictly-ordered queue: the DMA device is FIFO across
            # queues, so the critical stream must not interleave.
            nc.sync.dma_start(out=xT_sb[:, :, 0:128], in_=xt[:, :, 0:128])
            nc.sync.dma_start(out=wT_sb[:, 0:4, :], in_=wt[:, 0:4, :])
            nc.sync.dma_start(out=wT_sb[:, 4:8, :], in_=wt[:, 4:8, :])
            nc.sync.dma_start(out=tq_sb, in_=tq[:, :, :, :])
            if not shared_t:
                nc.sync.dma_start(out=tk_sb, in_=tk[:, :, :, :])
            nc.sync.dma_start(out=xT_sb[:, :, 128:640], in_=xt[:, :, 128:640])
            nc.sync.dma_start(out=id_sb, in_=ident[:, :])
            for c0, c1 in ((640, 1152), (1152, 1664), (1664, 2048)):
                nc.sync.dma_start(
                    out=xT_sb[:, :, c0:c1], in_=xt[:, :, c0:c1]
                )
            nc.sync.dma_start(out=pwT_sb, in_=pwt[:, :, :])
            if with_pb:
                pb_rep = persist.tile([128, C], F32)
                pb_ap = pb[:]
                nc.gpsimd.dma_start(
                    out=pb_rep,
                    in_=bass.AP(
                        tensor=pb_ap.tensor,
                        offset=pb_ap.offset,
                        ap=[[0, 128]] + list(pb_ap.ap),
                    ),
                )
            nc.vector.memset(v_sb[:, :, :, HD : HD + 1], 1.0)

            # ---- phase D: qkv matmul + LN + rope (transposes pipelined) --
            pend_t = []

            def flush_transpose():
                ii, qkr_i = pend_t.pop(0)
                tp = tpps.tile([128, 4, 128], F16, tag="tp")
                for hp in range(2):
                    nc.tensor.transpose(
                        tp[:, hp, :], qkr_i[:, 2 * hp : 2 * hp + 2, :], id_sb
                    )
                    nc.tensor.transpose(
                        tp[:, 2 + hp, :],
                        qkr_i[:, NH + 2 * hp : NH + 2 * hp + 2, :],
                        id_sb,
                    )
                # alternate the PSUM->SBUF eviction between DVE and ACT
                if ii % 2 == 0:
                    nc.vector.tensor_copy(
                        out=qT_sb[:, 0:2, ii * 128 : (ii + 1) * 128],
                        in_=tp[:, 0:2, :],
                    )
                    nc.scalar.copy(
                        out=kT_sb[:, 0:2, ii * 128 : (ii + 1) * 128],
                        in_=tp[:, 2:4, :],
                    )
                else:
                    nc.scalar.copy(
                        out=qT_sb[:, 0:2, ii * 128 : (ii + 1) * 128],
                        in_=tp[:, 0:2, :],
                    )
                    nc.vector.tensor_copy(
                        out=kT_sb[:, 0:2, ii * 128 : (ii + 1) * 128],
                        in_=tp[:, 2:4, :],
                    )

            for i in rng(NT, "D"):
                qp = qkvps.tile([128, 512], F32, tag="qp")
                # padded to the av-accumulator slot size (tag shared with
                # the attention phase's AV tiles)
                vp = qkvps.tile([128, 264], F32, tag="vp", padded_shape=[128, 390])
                for cc in range(CCH):
                    nc.tensor.matmul(
                        qp,
                        lhsT=xT_sb[:, cc, i * 128 : (i + 1) * 128],
                        rhs=wT_sb[:, cc, 0:512],
                        start=(cc == 0),
                        stop=(cc == CCH - 1),
                    )
                    nc.tensor.matmul(
                        vp,
                        lhsT=xT_sb[:, cc, i * 128 : (i + 1) * 128],
                        rhs=wT_sb[:, cc, 512:776],
                        start=(cc == 0),
                        stop=(cc == CCH - 1),
                    )
                # LN reads qp straight from PSUM (qp slots have ~3 tiles of
                # slack); sumsq/64 via ACT Square with scale=1/8 folded in,
                # f16 squares so the DVE reduce runs at 2 elem/cycle.
                qp3 = qp.rearrange("p (g d) -> p g d", g=NG)
                sq = dwork.tile([128, NG, HD], F16, tag="sq")
                sums = dwork.tile([128, 4, NG], F32, tag="sums")
                nc.scalar.activation(out=sq, in_=qp3, func=AF.Square, scale=0.125)
                # v eviction right after the Square: emitting it early keeps
                # the vp slot rotation off the LN chain's critical path.
                nc.vector.tensor_copy(
                    out=v_sb[:, i, :, 0:HD],
                    in_=vp[:, 0:256].rearrange("p (h d) -> p h d", h=NH),
                )
                nc.vector.tensor_reduce(
                    out=sums[:, 1, :], in_=sq, axis=AX.X, op=ALU.add
                )
                # var = ss/64 - mu^2 ; rstd = 1/sqrt(var + eps)
                nc.vector.tensor_copy(out=sums[:, 0, :], in_=vp[:, 256:264])
                nc.vector.tensor_mul(
                    out=sums[:, 2, :],
                    in0=sums[:, 0, :],
                    in1=sums[:, 0, :],
                )
                nc.vector.tensor_sub(
                    out=sums[:, 1, :], in0=sums[:, 1, :], in1=sums[:, 2, :]
                )
                nc.scalar.activation(
                    out=sums[:, 1, :], in_=sums[:, 1, :], func=AF.Sqrt, bias=EPS
                )
                nc.vector.reciprocal(out=sums[:, 1, :], in_=sums[:, 1, :])
                # nb = (-mu) * rstd
                nc.vector.tensor_mul(
                    out=sums[:, 2, :], in0=sums[:, 0, :], in1=sums[:, 1, :]
                )
                qk_sb = dwork.tile([128, NG, HD], F16, tag="qk")
                for g in range(NG):
                    if g < 4:
                        nc.scalar.activation(
                            out=qk_sb[:, g, :],
                            in_=qp3[:, g, :],
                            func=AF.Identity,
                            bias=sums[:, 2, g : g + 1],
                            scale=sums[:, 1, g : g + 1],
                        )
                    else:
                        nc.vector.tensor_scalar(
                            out=qk_sb[:, g, :],
                            in0=qp3[:, g, :],
                            scalar1=sums[:, 1, g : g + 1],
                            scalar2=sums[:, 2, g : g + 1],
                            op0=ALU.mult,
                            op1=ALU.add,
                        )
                # rope (tables carry the LN weights; broadcast over groups)
                qkr = dwork.tile([128, NG, HD], F16, tag="qkr")
                r = i % RB
                if shared_t:
                    groups = ((tq_sb, 0, NG, with_qb),)
                else:
                    groups = (
                        (tq_sb, 0, NH, with_qb),
                        (tk_sb, NH, NH, with_kb),
                    )
                t_full = dwork.tile([128, NG, 32], F16, tag="ropetmp")
                t_full2 = dwork.tile([128, NG, 32], F16, tag="ropetmp2")
                for tsb, base, gn, wb in groups:
                    a1 = qk_sb[:, base : base + gn, 0:32]
                    a2 = qk_sb[:, base : base + gn, 32:64]
                    o1 = qkr[:, base : base + gn, 0:32]
                    o2 = qkr[:, base : base + gn, 32:64]
                    t = t_full[:, base : base + gn, :]
                    t2 = t_full2[:, base : base + gn, :]
                    tb = lambda k: _bcast(tsb[:, r, k, :], gn)
                    # o1 chain on DVE, o2 chain on Pool (both SBUF f16)
                    nc.vector.tensor_mul(out=t, in0=a1, in1=tb(0))
                    nc.vector.tensor_mul(out=o1, in0=a2, in1=tb(1))
                    nc.gpsimd.tensor_sub(out=o1, in0=t, in1=o1)
                    nc.vector.tensor_mul(out=t2, in0=a2, in1=tb(2))
                    nc.gpsimd.tensor_mul(out=o2, in0=a1, in1=tb(3))
                    nc.gpsimd.tensor_add(out=o2, in0=t2, in1=o2)
                    if wb:
                        nc.vector.tensor_add(out=o1, in0=o1, in1=tb(4))
                        nc.gpsimd.tensor_add(out=o2, in0=o2, in1=tb(5))
                pend_t.append((i, qkr))
                if len(pend_t) > 4:
                    flush_transpose()

            # Switch the act table to the exp set now, overlapped with the
            # rope/transpose tail (the load runs after tile 15's Sqrt).
            nc.scalar.activation(out=cst[:, 3:4], in_=cst[:, 1:2], func=AF.Exp)
            if "E" not in PHASES:
                while pend_t:
                    flush_transpose()

            # ---- phase E: attention (scoresT -> exp -> AV^T) + proj -----
            ocnt = [0]

            def proj_tile(i):
                # y[i*128:(i+1)*128, :] = o^T pw (+pb), streamed as 2 halves
                for oc in range(2):
                    op = tpps.tile([128, 512], F32, tag="tp")
                    for cc in range(2):
                        nc.tensor.matmul(
                            op,
                            lhsT=oT_sb[:, cc, i * 128 : (i + 1) * 128],
                            rhs=pwT_sb[:, cc, oc * 512 : (oc + 1) * 512],
                            start=(cc == 0),
                            stop=(cc == 1),
                        )
                    ot = fwork.tile([128, 512], F16, tag="ot")
                    if with_pb:
                        # DVE both halves: Pool has no PSUM read port
                        nc.vector.tensor_add(
                            out=ot, in0=op, in1=pb_rep[:, oc * 512 : (oc + 1) * 512]
                        )
                    else:
                        if oc == 0:
                            nc.vector.tensor_copy(out=ot, in_=op)
                        else:
                            nc.scalar.copy(out=ot, in_=op)
                    q = (nc.scalar, nc.sync)[ocnt[0] % 2]
                    ocnt[0] += 1
                    q.dma_start(
                        out=out_p[i * 128 : (i + 1) * 128, oc * 512 : (oc + 1) * 512],
                        in_=ot,
                    )

            pend_norm = []

            def norm_qt(o32_, row_, qt_, nqh_, hp_):
                # o = av[:, 0:64] / av[:, 64] per head, from the
                # SBUF-evacuated accumulator; the multiply runs on Pool
                # (SBUF-only engine), then PE-transpose to (d, q).
                av2 = o32_[:, row_, :].rearrange("p (z t) -> p z t", z=2)
                rcp = nwork.tile([128, 2], F32, tag="rcp")
                o16 = nwork.tile([128, 128], F16, tag="o16")
                nc.vector.reciprocal(
                    out=rcp.rearrange("p (z o) -> p z o", z=2),
                    in_=av2[:, :, 64:65],
                )
                nc.gpsimd.tensor_mul(
                    out=o16.rearrange("p (z d) -> p z d", z=2),
                    in0=av2[:, :, 0:64],
                    in1=_bcast(rcp, 64, axis_insert=2),
                )
                tp2 = tpps.tile([128, 128], F16, tag="tp")
                nc.tensor.transpose(tp2, o16, id_sb)
                col = nqh_ * 1024 + qt_ * 128
                nc.vector.tensor_copy(out=oT_sb[:, hp_, col : col + 128], in_=tp2)

            for pi in rng(4, "E"):
                nqh, hp = PAIRS[pi]
                avt = [
                    qkvps.tile([128, 3, 130], F32, tag="vp", name=f"av{pi}_{j}")
                    for j in range(3)
                ]
                pend_av = []

                def av_flush(qts=range(8), pop=True):
                    kc_, e0_, e1_ = pend_av[0]
                    for qt in qts:
                        a = avt[qt // 3]
                        row = qt % 3
                        for z in range(2):
                            e = (e0_, e1_)[z]
                            # start=True zeroes the WHOLE 2KB psum bank, so
                            # only the first region-matmul of each bank may
                            # carry it; the others accumulate onto zeros.
                            first_of_bank = qt % 3 == 0 and z == 0
                            nc.tensor.matmul(
                                a[:, row, z * 65 : (z + 1) * 65],
                                lhsT=e[:, qt * 128 : (qt + 1) * 128],
                                rhs=v_sb[:, kc_, 2 * hp + z, :],
                                start=(kc_ == 0 and first_of_bank),
                                stop=(kc_ == NT - 1),
                            )
                    if pop:
                        pend_av.pop(0)

                # the SBUF evacuation at pair end decouples norms from the
                # av PSUM slots, so a short AV lag suffices
                lag = 2 if pi == 3 else 4
                for kc in range(NT):
                    bias = 0.0 if ((kc < 8) == (nqh == 0)) else LOG_COND
                    e0 = epool.tile([128, 1024], F16, tag="E", name="e0")
                    e1 = epool.tile([128, 1024], F16, tag="E", name="e1")
                    # the 4th score tile borrows the (mostly idle) tp slots,
                    # widening the sp recycle loop from 3 to 5 buffers —
                    # except where proj competes for them.
                    tp_ok = not (pi == 2 and kc >= 8) and not (pi == 3 and kc >= 13)
                    for z in range(2):
                        e = (e0, e1)[z]
                        for half in range(2):
                            tag = "tp" if (z == 1 and half == 1 and tp_ok) else "qp"
                            sp = qkvps.tile(
                                [128, 512], F32, tag=tag, name="sp"
                            ) if tag == "qp" else tpps.tile(
                                [128, 512], F32, tag="tp", name="sp"
                            )
                            q0 = nqh * 1024 + half * 512
                            nc.tensor.matmul(
                                sp,
                                lhsT=kT_sb[
                                    z * 64 : (z + 1) * 64,
                                    hp,
                                    kc * 128 : (kc + 1) * 128,
                                ],
                                rhs=qT_sb[z * 64 : (z + 1) * 64, hp, q0 : q0 + 512],
                                start=True,
                                stop=True,
                            )
                            # exp engine split: ACT (native exp) takes both
                            # first halves plus z1's second half every 4th
                            # kc (9:7 ACT:DVE); DVE takes the rest via the
                            # Schraudolph bit-trick.
                            on_act = half == 0 or (z == 1 and kc % 4 == 3)
                            if on_act:
                                nc.scalar.activation(
                                    out=e[:, half * 512 : (half + 1) * 512],
                                    in_=sp,
                                    func=AF.Exp,
                                    bias=bias,
                                    scale=SCALE,
                                )
                            else:
                                nc.vector.tensor_scalar(
                                    out=e[:, half * 512 : (half + 1) * 512].bitcast(I16),
                                    in0=sp,
                                    scalar1=SCALE * SCH_S,
                                    scalar2=bias * SCH_S + SCH_C,
                                    op0=ALU.mult,
                                    op1=ALU.add,
                                )
                    pend_av.append((kc, e0, e1))
                    if len(pend_av) > lag:
                        av_flush()
                    # leftover work from the previous phase/pair is emitted
                    # AFTER this kc's score/exp chain so the aux-engine
                    # queues never delay the sp-slot recycle.
                    if pi == 0 and kc in (1, 3, 5, 7) and pend_t:
                        flush_transpose()
                    if pend_norm and kc % 2 == 0:
                        pend_norm.pop(0)()
                    if pi == 2 and kc >= 8 and "F" in PHASES:
                        proj_tile(kc - 8)
                # drain, then evacuate the closed accumulators to SBUF in
                # three wide copies: the PSUM slots free immediately and the
                # norm pipeline becomes SBUF-only (spreadable anywhere).
                if pi < 3:
                    while pend_av:
                        av_flush()
                    o32s = []
                    for j in range(3):
                        o32 = nwork.tile(
                            [128, 3, 130], F32, tag="o32", name=f"o32_{pi}_{j}"
                        )
                        if j == 1:
                            nc.scalar.copy(out=o32, in_=avt[j])
                        else:
                            nc.vector.tensor_copy(out=o32, in_=avt[j])
                        o32s.append(o32)
                    pend_norm = [
                        (lambda o=o32s[qt // 3], r=qt % 3, q=qt, n=nqh, h=hp:
                         norm_qt(o, r, q, n, h))
                        for qt in range(8)
                    ]
                else:
                    while len(pend_av) > 1:
                        av_flush()
                    done_qt = 0
                    for gi, qte in enumerate((3, 6, 8)):
                        av_flush(qts=range(done_qt, qte), pop=(qte == 8))
                        o32 = nwork.tile([128, 3, 130], F32, tag="o32")
                        if gi == 1:
                            nc.scalar.copy(out=o32, in_=avt[gi])
                        else:
                            nc.vector.tensor_copy(out=o32, in_=avt[gi])
                        for qt in range(done_qt, qte):
                            norm_qt(o32, qt % 3, qt, nqh, hp)
                            if "F" in PHASES:
                                proj_tile(8 + qt)
                        done_qt = qte
            while pend_norm:
                pend_norm.pop(0)()
            if dbg is not None:
                nc.sync.dma_start(out=dbg["dbg_qT"][:, :, :], in_=qT_sb)
                nc.sync.dma_start(out=dbg["dbg_kT"][:, :, :], in_=kT_sb)
                nc.sync.dma_start(out=dbg["dbg_oT"][:, :, :], in_=oT_sb)
                nc.sync.dma_start(out=dbg["dbg_v"][:, :, :, :], in_=v_sb)

    nc.compile()
    return nc


def _rope_tables(n_w, n_b, with_b):
    inv = 1.0 / (10000.0 ** (np.arange(0, HD, 2, dtype=np.float64) / HD))
    ang = np.arange(NIN, dtype=np.float64)[:, None] * inv[None, :]  # (NIN, 32)
    cos_h = np.cos(ang)
    sin_h = np.sin(ang)
    w1, w2 = n_w[:32].astype(np.float64), n_w[32:].astype(np.float64)
    b1, b2 = n_b[:32].astype(np.float64), n_b[32:].astype(np.float64)
    kinds = [w1 * cos_h, w2 * sin_h, w2 * cos_h, w1 * sin_h]
    if with_b:
        kinds += [b1 * cos_h - b2 * sin_h, b2 * cos_h + b1 * sin_h]
    t = np.stack(kinds, axis=1)  # (NIN, k, 32)
    k = t.shape[1]
    return np.ascontiguousarray(
        t.astype(np.float16).reshape(RB, 128, k, 32).transpose(1, 0, 2, 3)
    )


_NC_CACHE = {}


def kernel(x, qkv_w, qn_w, qn_b, kn_w, kn_b, proj_w, proj_b):
    x = np.asarray(x, np.float32)
    qkv_w = np.asarray(qkv_w, np.float32)
    proj_w = np.asarray(proj_w, np.float32)
    proj_b = np.asarray(proj_b, np.float32)
    qn_w = np.asarray(qn_w, np.float32)
    qn_b = np.asarray(qn_b, np.float32)
    kn_w = np.asarray(kn_w, np.float32)
    kn_b = np.asarray(kn_b, np.float32)

    with_qb = bool(np.any(qn_b != 0))
    with_kb = bool(np.any(kn_b != 0))
    shared_t = (
        with_qb == with_kb
        and np.array_equal(qn_w, kn_w)
        and np.array_equal(qn_b, kn_b)
    )
    with_pb = bool(np.any(proj_b != 0))
    key = (with_qb, with_kb, shared_t, with_pb)
    if key not in _NC_CACHE:
        _NC_CACHE[key] = build_nc(with_qb, with_kb, shared_t, with_pb)
    nc = _NC_CACHE[key]

    tq = _rope_tables(qn_w, qn_b, with_qb)
    tk = None if shared_t else _rope_tables(kn_w, kn_b, with_kb)

    ident = np.eye(128, dtype=np.float16)

    in_maps = []
    for core in range(NCORES):
        b, g = core // GH, core % GH
        rows = slice(g * NH * HD, (g + 1) * NH * HD)
        # column layout: [q(256) k(256)] then [v(256) mq(4) mk(4)]
        wq = qkv_w[rows]
        wk = qkv_w[C:][rows]
        wv = qkv_w[2 * C :][rows]
        # negated so the matmul mean columns land as -mu (saves a negate op)
        means = -np.stack(
            [wq[h * HD : (h + 1) * HD].mean(0) for h in range(NH)]
            + [wk[h * HD : (h + 1) * HD].mean(0) for h in range(NH)]
        )  # (8, C)
        w_aug = np.concatenate([wq, wk, wv, means], 0)  # (776, C)
        wt16 = np.ascontiguousarray(
            w_aug.T.reshape(CCH, 128, JA).transpose(1, 0, 2).astype(np.float16)
        )
        xt16 = np.ascontiguousarray(
            x[b].T.reshape(CCH, 128, N).transpose(1, 0, 2).astype(np.float16)
        )
        pwt16 = np.ascontiguousarray(
            proj_w[:, rows].T.reshape(2, 128, C).transpose(1, 0, 2).astype(np.float16)
        )
        im = {
            "xt": xt16,
            "wt": wt16,
            "pwt": pwt16,
            "ident": ident,
            "tq": tq,
        }
        if with_pb:
            im["pb"] = proj_b if g == 0 else np.zeros_like(proj_b)
        if tk is not None:
            im["tk"] = tk
        in_maps.append(im)

    res = bass_utils.run_bass_kernel_spmd(nc, in_maps, core_ids=list(range(NCORES)))
    parts = [r["out_p"].astype(np.float32) for r in res.results]
    out = np.stack(
        [np.sum(parts[b * GH : (b + 1) * GH], axis=0) for b in range(B)]
    )
    return out.astype(np.float32)


if __name__ == "__main__":
    rng = np.random.default_rng(0)
    ins = {
        "x": rng.standard_normal((B, N, C), np.float32),
        "qkv_w": (rng.standard_normal((3 * C, C), np.float32) / math.sqrt(C)).astype(
            np.float32
        ),
        "qn_w": np.ones(HD, np.float32),
        "qn_b": np.zeros(HD, np.float32),
        "kn_w": np.ones(HD, np.float32),
        "kn_b": np.zeros(HD, np.float32),
        "proj_w": (rng.standard_normal((C, C), np.float32) / math.sqrt(C)).astype(
            np.float32
        ),
        "proj_b": np.zeros(C, np.float32),
    }
    o = kernel(**ins)
    print(o.shape, o.dtype)


# revision 101
# speedup vs baseline: 1.0217x; 1.0217x over previous
"""Trainium2 Bass kernel for nn_Attention_66949950210549.

Dense transformer attention block:
  qkv = x @ qkv_w.T ; per-head LN on q,k ; RoPE (positions restart at N/2) ;
  softmax(q k^T * HD^-0.5 + cross-block log(0.5) bias) @ v ; proj.

Sharding: 8 cores = 2 (batch) x 4 (head groups of 4 heads).  Each core
computes its batch's qkv for its 4 heads, attention, and a partial
projection (row-parallel over the head channels); the host sums the 4
partials per batch in f32 (partials stored f16).

v5 design notes (cost-model driven; 235.8us -> 187.3us):
  - matmul cost on PE is proportional to the OUTPUT free size only, so AV
    runs transposed: out[q=128, d0..d63, denom] = e^T v per (head, q-tile,
    k-chunk) costs 65 cycles/chunk instead of 512 in the (d, q) layout.
    The softmax denominator accumulates in column 64 via a ones-column on
    v; normalization is a per-partition reciprocal+scale (no partition
    broadcast, no DRAM bounce).  o is PE-transposed per q-tile into (d, q)
    layout for the projection.
  - ONE set of PSUM pools is shared by all phases (sp tiles reuse the qkv
    qp slots, AV accumulators reuse the vp slots, proj/transposes reuse
    the tp slots), so there is no pool barrier at the phase boundary; a
    4th score tile per kc borrows the tp slots to widen the sp recycle
    loop (PE otherwise rate-locks to exp completions).
  - closed AV accumulators are EVACUATED to SBUF in three wide copies at
    pair end: the PSUM slots free immediately (short AV lag, no WAR
    coupling to the next pair) and the normalize multiplies move to the
    otherwise-idle Pool engine (SBUF-only access).
  - the last 4 qT/kT transposes are interleaved into the first attention
    pair's kc loop; deferred normalizes of each pair spread 1-per-kc into
    the next pair.  Per-tile v eviction is emitted right after the Square
    so the vp slot rotation stays off the LN chain's critical path.
  - act tables: a dummy Sqrt at t=0 pins the sqrt set under the initial
    DMA shadow; a dummy Exp after the last LN Sqrt switches sets during
    the rope tail so the first real softmax exp is never table-stalled.
  - exp is engine-split per kc (ACT 2.25 halves native Exp, DVE 1.75
    halves Schraudolph bit-trick i16 = rint(x*2^10/ln2 + (15*2^10-44.5)),
    bits as fp16); proj output copies split ACT/DVE.
  - all inputs are partition-major so each is one or two large contiguous
    DMAs (the DMA queues pay ~0.7us dispatch per transfer and the DMA
    device is FIFO across queues), streamed on one strictly-ordered
    queue in consumption order.
"""

import math
import os
import sys

sys.path.insert(0, "/opt/trn_rl_repo")

PHASES = os.environ.get("BASS_PHASES", "DEF")

import numpy as np

import concourse.bacc as bacc
import concourse.bass as bass
import concourse.tile as tile
from concourse import bass_utils, mybir

B, N, C = 2, 2048, 1024
H, HD = 16, 64
NCORES = 8
GH = 4  # head-group count (cores per batch)
NH = H // GH  # heads per core = 4
NG = 2 * NH  # LN groups per core (q heads + k heads) = 8
J = 3 * NH * HD  # qkv rows per core = 768
JA = J + NG  # + 8 mean columns = 776
NIN = N // 2  # rope positions restart here
NT = N // 128  # 16 row tiles
CCH = C // 128  # 8 contraction chunks
RB = NIN // 128  # rope table row blocks = 8
LOG_COND = math.log(0.5)
EPS = 1e-5
SCALE = HD ** -0.5  # 0.125

# Schraudolph fp16 exp: i16 = rint(x * SCH_S + SCH_C); bits as fp16.
SCH_S = float(2 ** 10) / math.log(2.0)
SCH_C = 15.0 * 2 ** 10 - 44.5

F32 = mybir.dt.float32
F16 = mybir.dt.float16
I16 = mybir.dt.int16
AF = mybir.ActivationFunctionType
AX = mybir.AxisListType
ALU = mybir.AluOpType

PAIRS = [(0, 0), (0, 1), (1, 0), (1, 1)]  # (nqh, hp)


def _bcast(ap, n, axis_insert=1):
    """Insert a 0-stride dim of size n into an AP (free-dim broadcast)."""
    new = list(ap.ap)
    new.insert(axis_insert, [0, n])
    return bass.AP(tensor=ap.tensor, offset=ap.offset, ap=new)


def build_nc(with_qb: bool, with_kb: bool, shared_t: bool = False, with_pb: bool = True):
    nc = bacc.Bacc("TRN2", target_bir_lowering=False, debug=False)

    # all inputs partition-major so each is one or two large contiguous
    # DMAs (the DMA queues pay ~0.7-1.3us dispatch latency per transfer)
    xt = nc.dram_tensor("xt", [128, CCH, N], F16, kind="ExternalInput")
    wt = nc.dram_tensor("wt", [128, CCH, JA], F16, kind="ExternalInput")
    pwt = nc.dram_tensor("pwt", [128, 2, C], F16, kind="ExternalInput")
    ident = nc.dram_tensor("ident", [128, 128], F16, kind="ExternalInput")
    nkinds_q = 6 if with_qb else 4
    nkinds_k = 6 if with_kb else 4
    tq = nc.dram_tensor("tq", [128, RB, nkinds_q, 32], F16, kind="ExternalInput")
    tk = None
    if not shared_t:
        tk = nc.dram_tensor("tk", [128, RB, nkinds_k, 32], F16, kind="ExternalInput")
    pb = None
    if with_pb:
        pb = nc.dram_tensor("pb", [C], F32, kind="ExternalInput")
    out_p = nc.dram_tensor("out_p", [N, C], F16, kind="ExternalOutput")
    dbg = None
    if os.environ.get("BASS_DEBUG"):
        dbg = {
            "dbg_qT": nc.dram_tensor("dbg_qT", [128, 2, N], F16, kind="ExternalOutput"),
            "dbg_kT": nc.dram_tensor("dbg_kT", [128, 2, N], F16, kind="ExternalOutput"),
            "dbg_oT": nc.dram_tensor("dbg_oT", [128, 2, N], F16, kind="ExternalOutput"),
            "dbg_v": nc.dram_tensor(
                "dbg_v", [128, NT, NH, HD + 1], F16, kind="ExternalOutput"
            ),
        }

    def rng(n, ph):
        return range(n if ph in PHASES else 0)

    with tile.TileContext(nc) as tc:
        with (
            tc.tile_pool(name="persist", bufs=1) as persist,
            tc.tile_pool(name="qkvps", bufs=3, space="PSUM") as qkvps,
            tc.tile_pool(name="tpps", bufs=2, space="PSUM") as tpps,
            tc.tile_pool(name="dwork", bufs=8) as dwork,
            tc.tile_pool(name="epool", bufs=32) as epool,
            tc.tile_pool(name="nwork", bufs=8) as nwork,
            tc.tile_pool(name="fwork", bufs=8) as fwork,
        ):
            # ---- persistent SBUF tensors --------------------------------
            xT_sb = persist.tile([128, CCH, N], F16)
            wT_sb = persist.tile([128, CCH, JA], F16)
            pwT_sb = persist.tile([128, 2, C], F16)
            v_sb = persist.tile([128, NT, NH, HD + 1], F16)
            qT_sb = persist.tile([128, 2, N], F16)
            kT_sb = persist.tile([128, 2, N], F16)
            oT_sb = persist.tile([128, 2, N], F16)
            tq_sb = persist.tile([128, RB, nkinds_q, 32], F16)
            id_sb = persist.tile([128, 128], F16)
            tk_sb = None
            if not shared_t:
                tk_sb = persist.tile([128, RB, nkinds_k, 32], F16)
            pb_rep = None

            cst = persist.tile([128, 4], F32)
            nc.vector.memset(cst[:, 0:1], EPS)
            nc.vector.memset(cst[:, 1:2], 0.0)
            nc.vector.memset(cst[:, 2:3], LOG_COND)
            nc.const_aps.aps[(F32, EPS)] = cst[:, 0:1]
            nc.const_aps.aps[(F32, 0.0)] = cst[:, 1:2]
            nc.const_aps.aps[(F32, LOG_COND)] = cst[:, 2:3]
            # Pin the sqrt act-table at t=0 (under the DMA shadow) so the
            # per-tile LN Sqrt never stalls on a table load.
            nc.scalar.activation(out=cst[:, 3:4], in_=cst[:, 0:1], func=AF.Sqrt)

            # ---- input DMAs in strict consumption order -----------------
            # sync queue carries the critical stream: half the weights, the
            # leading x columns, the rest of the weights, then x column
            # blocks in tile order, proj weights last.
            # single strictly-ordered queue: the DMA device is FIFO across
            # queues, so the critical stream must not interleave.
            nc.sync.dma_start(out=xT_sb[:, :, 0:128], in_=xt[:, :, 0:128])
            nc.sync.dma_start(out=wT_sb[:, 0:4, :], in_=wt[:, 0:4, :])
            nc.sync.dma_start(out=wT_sb[:, 4:8, :], in_=wt[:, 4:8, :])
            nc.sync.dma_start(out=tq_sb, in_=tq[:, :, :, :])
            if not shared_t:
                nc.sync.dma_start(out=tk_sb, in_=tk[:, :, :, :])
            nc.sync.dma_start(out=xT_sb[:, :, 128:640], in_=xt[:, :, 128:640])
            nc.sync.dma_start(out=id_sb, in_=ident[:, :])
            for c0, c1 in ((640, 1152), (1152, 1664), (1664, 2048)):
                nc.sync.dma_start(
                    out=xT_sb[:, :, c0:c1], in_=xt[:, :, c0:c1]
                )
            nc.sync.dma_start(out=pwT_sb, in_=pwt[:, :, :])
            if with_pb:
                pb_rep = persist.tile([128, C], F32)
                pb_ap = pb[:]
                nc.gpsimd.dma_start(
                    out=pb_rep,
                    in_=bass.AP(
                        tensor=pb_ap.tensor,
                        offset=pb_ap.offset,
                        ap=[[0, 128]] + list(pb_ap.ap),
                    ),
                )
            nc.vector.memset(v_sb[:, :, :, HD : HD + 1], 1.0)

            # ---- phase D: qkv matmul + LN + rope (transposes pipelined) --
            pend_t = []

            def flush_transpose():
                ii, qkr_i = pend_t.pop(0)
                tp = tpps.tile([128, 4, 128], F16, tag="tp")
                for hp in range(2):
                    nc.tensor.transpose(
                        tp[:, hp, :], qkr_i[:, 2 * hp : 2 * hp + 2, :], id_sb
                    )
                    nc.tensor.transpose(
                        tp[:, 2 + hp, :],
                        qkr_i[:, NH + 2 * hp : NH + 2 * hp + 2, :],
                        id_sb,
                    )
                # alternate the PSUM->SBUF eviction between DVE and ACT
                if ii % 2 == 0:
                    nc.vector.tensor_copy(
                        out=qT_sb[:, 0:2, ii * 128 : (ii + 1) * 128],
                        in_=tp[:, 0:2, :],
                    )
                    nc.scalar.copy(
                        out=kT_sb[:, 0:2, ii * 128 : (ii + 1) * 128],
                        in_=tp[:, 2:4, :],
                    )
                else:
                    nc.scalar.copy(
                        out=qT_sb[:, 0:2, ii * 128 : (ii + 1) * 128],
                        in_=tp[:, 0:2, :],
                    )
                    nc.vector.tensor_copy(
                        out=kT_sb[:, 0:2, ii * 128 : (ii + 1) * 128],
                        in_=tp[:, 2:4, :],
                    )

            for i in rng(NT, "D"):
                qp = qkvps.tile([128, 512], F32, tag="qp")
                # padded to the av-accumulator slot size (tag shared with
                # the attention phase's AV tiles)
                vp = qkvps.tile([128, 264], F32, tag="vp", padded_shape=[128, 390])
                for cc in range(CCH):
                    nc.tensor.matmul(
                        qp,
                        lhsT=xT_sb[:, cc, i * 128 : (i + 1) * 128],
                        rhs=wT_sb[:, cc, 0:512],
                        start=(cc == 0),
                        stop=(cc == CCH - 1),
                    )
                    nc.tensor.matmul(
                        vp,
                        lhsT=xT_sb[:, cc, i * 128 : (i + 1) * 128],
                        rhs=wT_sb[:, cc, 512:776],
                        start=(cc == 0),
                        stop=(cc == CCH - 1),
                    )
                # LN reads qp straight from PSUM (qp slots have ~3 tiles of
                # slack); sumsq/64 via ACT Square with scale=1/8 folded in,
                # f16 squares so the DVE reduce runs at 2 elem/cycle.
                qp3 = qp.rearrange("p (g d) -> p g d", g=NG)
                sq = dwork.tile([128, NG, HD], F16, tag="sq")
                sums = dwork.tile([128, 4, NG], F32, tag="sums")
                nc.scalar.activation(out=sq, in_=qp3, func=AF.Square, scale=0.125)
                # v eviction right after the Square: emitting it early keeps
                # the vp slot rotation off the LN chain's critical path.
                nc.scalar.copy(
                    out=v_sb[:, i, :, 0:HD],
                    in_=vp[:, 0:256].rearrange("p (h d) -> p h d", h=NH),
                )
                nc.vector.tensor_reduce(
                    out=sums[:, 1, :], in_=sq, axis=AX.X, op=ALU.add
                )
                # var = ss/64 - mu^2 ; rstd = 1/sqrt(var + eps)
                nc.vector.tensor_copy(out=sums[:, 0, :], in_=vp[:, 256:264])
                nc.vector.tensor_mul(
                    out=sums[:, 2, :],
                    in0=sums[:, 0, :],
                    in1=sums[:, 0, :],
                )
                nc.vector.tensor_sub(
                    out=sums[:, 1, :], in0=sums[:, 1, :], in1=sums[:, 2, :]
                )
                nc.scalar.activation(
                    out=sums[:, 1, :], in_=sums[:, 1, :], func=AF.Sqrt, bias=EPS
                )
                nc.vector.reciprocal(out=sums[:, 1, :], in_=sums[:, 1, :])
                # nb = (-mu) * rstd
                nc.vector.tensor_mul(
                    out=sums[:, 2, :], in0=sums[:, 0, :], in1=sums[:, 1, :]
                )
                qk_sb = dwork.tile([128, NG, HD], F16, tag="qk")
                for g in range(NG):
                    if g < 4:
                        nc.scalar.activation(
                            out=qk_sb[:, g, :],
                            in_=qp3[:, g, :],
                            func=AF.Identity,
                            bias=sums[:, 2, g : g + 1],
                            scale=sums[:, 1, g : g + 1],
                        )
                    else:
                        nc.vector.tensor_scalar(
                            out=qk_sb[:, g, :],
                            in0=qp3[:, g, :],
                            scalar1=sums[:, 1, g : g + 1],
                            scalar2=sums[:, 2, g : g + 1],
                            op0=ALU.mult,
                            op1=ALU.add,
                        )
                # rope (tables carry the LN weights; broadcast over groups)
                qkr = dwork.tile([128, NG, HD], F16, tag="qkr")
                r = i % RB
                if shared_t:
                    groups = ((tq_sb, 0, NG, with_qb),)
                else:
                    groups = (
                        (tq_sb, 0, NH, with_qb),
                        (tk_sb, NH, NH, with_kb),
                    )
                t_full = dwork.tile([128, NG, 32], F16, tag="ropetmp")
                t_full2 = dwork.tile([128, NG, 32], F16, tag="ropetmp2")
                for tsb, base, gn, wb in groups:
                    a1 = qk_sb[:, base : base + gn, 0:32]
                    a2 = qk_sb[:, base : base + gn, 32:64]
                    o1 = qkr[:, base : base + gn, 0:32]
                    o2 = qkr[:, base : base + gn, 32:64]
                    t = t_full[:, base : base + gn, :]
                    t2 = t_full2[:, base : base + gn, :]
                    tb = lambda k: _bcast(tsb[:, r, k, :], gn)
                    # o1 chain on DVE, o2 chain on Pool (both SBUF f16)
                    nc.vector.tensor_mul(out=t, in0=a1, in1=tb(0))
                    nc.vector.tensor_mul(out=o1, in0=a2, in1=tb(1))
                    nc.gpsimd.tensor_sub(out=o1, in0=t, in1=o1)
                    nc.vector.tensor_mul(out=t2, in0=a2, in1=tb(2))
                    nc.gpsimd.tensor_mul(out=o2, in0=a1, in1=tb(3))
                    nc.gpsimd.tensor_add(out=o2, in0=t2, in1=o2)
                    if wb:
                        nc.vector.tensor_add(out=o1, in0=o1, in1=tb(4))
                        nc.gpsimd.tensor_add(out=o2, in0=o2, in1=tb(5))
                pend_t.append((i, qkr))
                if len(pend_t) > 4:
                    flush_transpose()

            # Switch the act table to the exp set now, overlapped with the
            # rope/transpose tail (the load runs after tile 15's Sqrt).
            nc.scalar.activation(out=cst[:, 3:4], in_=cst[:, 1:2], func=AF.Exp)
            if "E" not in PHASES:
                while pend_t:
                    flush_transpose()

            # ---- phase E: attention (scoresT -> exp -> AV^T) + proj -----
            ocnt = [0]

            def proj_tile(i):
                # y[i*128:(i+1)*128, :] = o^T pw (+pb), streamed as 2 halves
                for oc in range(2):
                    op = tpps.tile([128, 512], F32, tag="tp")
                    for cc in range(2):
                        nc.tensor.matmul(
                            op,
                            lhsT=oT_sb[:, cc, i * 128 : (i + 1) * 128],
                            rhs=pwT_sb[:, cc, oc * 512 : (oc + 1) * 512],
                            start=(cc == 0),
                            stop=(cc == 1),
                        )
                    ot = fwork.tile([128, 512], F16, tag="ot")
                    if with_pb:
                        # DVE both halves: Pool has no PSUM read port
                        nc.vector.tensor_add(
                            out=ot, in0=op, in1=pb_rep[:, oc * 512 : (oc + 1) * 512]
                        )
                    else:
                        if oc == 0:
                            nc.vector.tensor_copy(out=ot, in_=op)
                        else:
                            nc.scalar.copy(out=ot, in_=op)
                    q = (nc.scalar, nc.sync)[ocnt[0] % 2]
                    ocnt[0] += 1
                    q.dma_start(
                        out=out_p[i * 128 : (i + 1) * 128, oc * 512 : (oc + 1) * 512],
                        in_=ot,
                    )

            pend_norm = []

            def norm_qt(o32_, row_, qt_, nqh_, hp_):
                # o = av[:, 0:64] / av[:, 64] per head, from the
                # SBUF-evacuated accumulator; the multiply runs on Pool
                # (SBUF-only engine), then PE-transpose to (d, q).
                av2 = o32_[:, row_, :].rearrange("p (z t) -> p z t", z=2)
                rcp = nwork.tile([128, 2], F32, tag="rcp")
                o16 = nwork.tile([128, 128], F16, tag="o16")
                nc.vector.reciprocal(
                    out=rcp.rearrange("p (z o) -> p z o", z=2),
                    in_=av2[:, :, 64:65],
                )
                nc.gpsimd.tensor_mul(
                    out=o16.rearrange("p (z d) -> p z d", z=2),
                    in0=av2[:, :, 0:64],
                    in1=_bcast(rcp, 64, axis_insert=2),
                )
                tp2 = tpps.tile([128, 128], F16, tag="tp")
                nc.tensor.transpose(tp2, o16, id_sb)
                col = nqh_ * 1024 + qt_ * 128
                nc.vector.tensor_copy(out=oT_sb[:, hp_, col : col + 128], in_=tp2)

            for pi in rng(4, "E"):
                nqh, hp = PAIRS[pi]
                avt = [
                    qkvps.tile([128, 3, 130], F32, tag="vp", name=f"av{pi}_{j}")
                    for j in range(3)
                ]
                pend_av = []

                def av_flush(qts=range(8), pop=True):
                    kc_, e0_, e1_ = pend_av[0]
                    for qt in qts:
                        a = avt[qt // 3]
                        row = qt % 3
                        for z in range(2):
                            e = (e0_, e1_)[z]
                            # start=True zeroes the WHOLE 2KB psum bank, so
                            # only the first region-matmul of each bank may
                            # carry it; the others accumulate onto zeros.
                            first_of_bank = qt % 3 == 0 and z == 0
                            nc.tensor.matmul(
                                a[:, row, z * 65 : (z + 1) * 65],
                                lhsT=e[:, qt * 128 : (qt + 1) * 128],
                                rhs=v_sb[:, kc_, 2 * hp + z, :],
                                start=(kc_ == 0 and first_of_bank),
                                stop=(kc_ == NT - 1),
                            )
                    if pop:
                        pend_av.pop(0)

                # the SBUF evacuation at pair end decouples norms from the
                # av PSUM slots, so a short AV lag suffices
                lag = 2 if pi == 3 else 4
                for kc in range(NT):
                    bias = 0.0 if ((kc < 8) == (nqh == 0)) else LOG_COND
                    e0 = epool.tile([128, 1024], F16, tag="E", name="e0")
                    e1 = epool.tile([128, 1024], F16, tag="E", name="e1")
                    # the 4th score tile borrows the (mostly idle) tp slots,
                    # widening the sp recycle loop from 3 to 5 buffers —
                    # except where proj competes for them.
                    tp_ok = not (pi == 2 and kc >= 8) and not (pi == 3 and kc >= 13)
                    for z in range(2):
                        e = (e0, e1)[z]
                        for half in range(2):
                            tag = "tp" if (z == 1 and half == 1 and tp_ok) else "qp"
                            sp = qkvps.tile(
                                [128, 512], F32, tag=tag, name="sp"
                            ) if tag == "qp" else tpps.tile(
                                [128, 512], F32, tag="tp", name="sp"
                            )
                            q0 = nqh * 1024 + half * 512
                            nc.tensor.matmul(
                                sp,
                                lhsT=kT_sb[
                                    z * 64 : (z + 1) * 64,
                                    hp,
                                    kc * 128 : (kc + 1) * 128,
                                ],
                                rhs=qT_sb[z * 64 : (z + 1) * 64, hp, q0 : q0 + 512],
                                start=True,
                                stop=True,
                            )
                            # exp engine split: ACT (native exp) takes both
                            # first halves plus z1's second half every 4th
                            # kc (9:7 ACT:DVE); DVE takes the rest via the
                            # Schraudolph bit-trick.
                            on_act = half == 0 or (z == 1 and kc % 4 == 3)
                            if on_act:
                                nc.scalar.activation(
                                    out=e[:, half * 512 : (half + 1) * 512],
                                    in_=sp,
                                    func=AF.Exp,
                                    bias=bias,
                                    scale=SCALE,
                                )
                            else:
                                nc.vector.tensor_scalar(
                                    out=e[:, half * 512 : (half + 1) * 512].bitcast(I16),
                                    in0=sp,
                                    scalar1=SCALE * SCH_S,
                                    scalar2=bias * SCH_S + SCH_C,
                                    op0=ALU.mult,
                                    op1=ALU.add,
                                )
                    pend_av.append((kc, e0, e1))
                    if len(pend_av) > lag:
                        av_flush()
                    # leftover work from the previous phase/pair is emitted
                    # AFTER this kc's score/exp chain so the aux-engine
                    # queues never delay the sp-slot recycle.
                    if pi == 0 and kc in (1, 3, 5, 7) and pend_t:
                        flush_transpose()
                    if pend_norm and kc % 2 == 0:
                        pend_norm.pop(0)()
                    if pi == 2 and kc >= 8 and "F" in PHASES:
                        proj_tile(kc - 8)
                # drain, then evacuate the closed accumulators to SBUF in
                # three wide copies: the PSUM slots free immediately and the
                # norm pipeline becomes SBUF-only (spreadable anywhere).
                if pi < 3:
                    while pend_av:
                        av_flush()
                    o32s = []
                    for j in range(3):
                        o32 = nwork.tile(
                            [128, 3, 130], F32, tag="o32", name=f"o32_{pi}_{j}"
                        )
                        if j == 1:
                            nc.scalar.copy(out=o32, in_=avt[j])
                        else:
                            nc.vector.tensor_copy(out=o32, in_=avt[j])
                        o32s.append(o32)
                    pend_norm = [
                        (lambda o=o32s[qt // 3], r=qt % 3, q=qt, n=nqh, h=hp:
                         norm_qt(o, r, q, n, h))
                        for qt in range(8)
                    ]
                else:
                    while len(pend_av) > 1:
                        av_flush()
                    done_qt = 0
                    for gi, qte in enumerate((3, 6, 8)):
                        av_flush(qts=range(done_qt, qte), pop=(qte == 8))
                        o32 = nwork.tile([128, 3, 130], F32, tag="o32")
                        if gi == 1:
                            nc.scalar.copy(out=o32, in_=avt[gi])
                        else:
                            nc.vector.tensor_copy(out=o32, in_=avt[gi])
                        for qt in range(done_qt, qte):
                            norm_qt(o32, qt % 3, qt, nqh, hp)
                            if "F" in PHASES:
                                proj_tile(8 + qt)
                        done_qt = qte
            while pend_norm:
                pend_norm.pop(0)()
            if dbg is not None:
                nc.sync.dma_start(out=dbg["dbg_qT"][:, :, :], in_=qT_sb)
                nc.sync.dma_start(out=dbg["dbg_kT"][:, :, :], in_=kT_sb)
                nc.sync.dma_start(out=dbg["dbg_oT"][:, :, :], in_=oT_sb)
                nc.sync.dma_start(out=dbg["dbg_v"][:, :, :, :], in_=v_sb)

    nc.compile()
    return nc


def _rope_tables(n_w, n_b, with_b):
    inv = 1.0 / (10000.0 ** (np.arange(0, HD, 2, dtype=np.float64) / HD))
    ang = np.arange(NIN, dtype=np.float64)[:, None] * inv[None, :]  # (NIN, 32)
    cos_h = np.cos(ang)
    sin_h = np.sin(ang)
    w1, w2 = n_w[:32].astype(np.float64), n_w[32:].astype(np.float64)
    b1, b2 = n_b[:32].astype(np.float64), n_b[32:].astype(np.float64)
    kinds = [w1 * cos_h, w2 * sin_h, w2 * cos_h, w1 * sin_h]
    if with_b:
        kinds += [b1 * cos_h - b2 * sin_h, b2 * cos_h + b1 * sin_h]
    t = np.stack(kinds, axis=1)  # (NIN, k, 32)
    k = t.shape[1]
    return np.ascontiguousarray(
        t.astype(np.float16).reshape(RB, 128, k, 32).transpose(1, 0, 2, 3)
    )


_NC_CACHE = {}


def kernel(x, qkv_w, qn_w, qn_b, kn_w, kn_b, proj_w, proj_b):
    x = np.asarray(x, np.float32)
    qkv_w = np.asarray(qkv_w, np.float32)
    proj_w = np.asarray(proj_w, np.float32)
    proj_b = np.asarray(proj_b, np.float32)
    qn_w = np.asarray(qn_w, np.float32)
    qn_b = np.asarray(qn_b, np.float32)
    kn_w = np.asarray(kn_w, np.float32)
    kn_b = np.asarray(kn_b, np.float32)

    with_qb = bool(np.any(qn_b != 0))
    with_kb = bool(np.any(kn_b != 0))
    shared_t = (
        with_qb == with_kb
        and np.array_equal(qn_w, kn_w)
        and np.array_equal(qn_b, kn_b)
    )
    with_pb = bool(np.any(proj_b != 0))
    key = (with_qb, with_kb, shared_t, with_pb)
    if key not in _NC_CACHE:
        _NC_CACHE[key] = build_nc(with_qb, with_kb, shared_t, with_pb)
    nc = _NC_CACHE[key]

    tq = _rope_tables(qn_w, qn_b, with_qb)
    tk = None if shared_t else _rope_tables(kn_w, kn_b, with_kb)

    ident = np.eye(128, dtype=np.float16)

    in_maps = []
    for core in range(NCORES):
        b, g = core // GH, core % GH
        rows = slice(g * NH * HD, (g + 1) * NH * HD)
        # column layout: [q(256) k(256)] then [v(256) mq(4) mk(4)]
        wq = qkv_w[rows]
        wk = qkv_w[C:][rows]
        wv = qkv_w[2 * C :][rows]
        # negated so the matmul mean columns land as -mu (saves a negate op)
        means = -np.stack(
            [wq[h * HD : (h + 1) * HD].mean(0) for h in range(NH)]
            + [wk[h * HD : (h + 1) * HD].mean(0) for h in range(NH)]
        )  # (8, C)
        w_aug = np.concatenate([wq, wk, wv, means], 0)  # (776, C)
        wt16 = np.ascontiguousarray(
            w_aug.T.reshape(CCH, 128, JA).transpose(1, 0, 2).astype(np.float16)
        )
        xt16 = np.ascontiguousarray(
            x[b].T.reshape(CCH, 128, N).transpose(1, 0, 2).astype(np.float16)
        )
        pwt16 = np.ascontiguousarray(
            proj_w[:, rows].T.reshape(2, 128, C).transpose(1, 0, 2).astype(np.float16)
        )
        im = {
            "xt": xt16,
            "wt": wt16,
            "pwt": pwt16,
            "ident": ident,
            "tq": tq,
        }
        if with_pb:
            im["pb"] = proj_b if g == 0 else np.zeros_like(proj_b)
        if tk is not None:
            im["tk"] = tk
        in_maps.append(im)

    res = bass_utils.run_bass_kernel_spmd(nc, in_maps, core_ids=list(range(NCORES)))
    parts = [r["out_p"].astype(np.float32) for r in res.results]
    out = np.stack(
        [np.sum(parts[b * GH : (b + 1) * GH], axis=0) for b in range(B)]
    )
    return out.astype(np.float32)


if __name__ == "__main__":
    rng = np.random.default_rng(0)
    ins = {
        "x": rng.standard_normal((B, N, C), np.float32),
        "qkv_w": (rng.standard_normal((3 * C, C), np.float32) / math.sqrt(C)).astype(
            np.float32
        ),
        "qn_w": np.ones(HD, np.float32),
        "qn_b": np.zeros(HD, np.float32),
        "kn_w": np.ones(HD, np.float32),
        "kn_b": np.zeros(HD, np.float32),
        "proj_w": (rng.standard_normal((C, C), np.float32) / math.sqrt(C)).astype(
            np.float32
        ),
        "proj_b": np.zeros(C, np.float32),
    }
    o = kernel(**ins)
    print(o.shape, o.dtype)


# revision 102
# speedup vs baseline: 1.0234x; 1.0017x over previous
"""Trainium2 Bass kernel for nn_Attention_66949950210549.

Dense transformer attention block:
  qkv = x @ qkv_w.T ; per-head LN on q,k ; RoPE (positions restart at N/2) ;
  softmax(q k^T * HD^-0.5 + cross-block log(0.5) bias) @ v ; proj.

Sharding: 8 cores = 2 (batch) x 4 (head groups of 4 heads).  Each core
computes its batch's qkv for its 4 heads, attention, and a partial
projection (row-parallel over the head channels); the host sums the 4
partials per batch in f32 (partials stored f16).

v5 design notes (cost-model driven; 235.8us -> 187.3us):
  - matmul cost on PE is proportional to the OUTPUT free size only, so AV
    runs transposed: out[q=128, d0..d63, denom] = e^T v per (head, q-tile,
    k-chunk) costs 65 cycles/chunk instead of 512 in the (d, q) layout.
    The softmax denominator accumulates in column 64 via a ones-column on
    v; normalization is a per-partition reciprocal+scale (no partition
    broadcast, no DRAM bounce).  o is PE-transposed per q-tile into (d, q)
    layout for the projection.
  - ONE set of PSUM pools is shared by all phases (sp tiles reuse the qkv
    qp slots, AV accumulators reuse the vp slots, proj/transposes reuse
    the tp slots), so there is no pool barrier at the phase boundary; a
    4th score tile per kc borrows the tp slots to widen the sp recycle
    loop (PE otherwise rate-locks to exp completions).
  - closed AV accumulators are EVACUATED to SBUF in three wide copies at
    pair end: the PSUM slots free immediately (short AV lag, no WAR
    coupling to the next pair) and the normalize multiplies move to the
    otherwise-idle Pool engine (SBUF-only access).
  - the last 4 qT/kT transposes are interleaved into the first attention
    pair's kc loop; deferred normalizes of each pair spread 1-per-kc into
    the next pair.  Per-tile v eviction is emitted right after the Square
    so the vp slot rotation stays off the LN chain's critical path.
  - act tables: a dummy Sqrt at t=0 pins the sqrt set under the initial
    DMA shadow; a dummy Exp after the last LN Sqrt switches sets during
    the rope tail so the first real softmax exp is never table-stalled.
  - exp is engine-split per kc (ACT 2.25 halves native Exp, DVE 1.75
    halves Schraudolph bit-trick i16 = rint(x*2^10/ln2 + (15*2^10-44.5)),
    bits as fp16); proj output copies split ACT/DVE.
  - all inputs are partition-major so each is one or two large contiguous
    DMAs (the DMA queues pay ~0.7us dispatch per transfer and the DMA
    device is FIFO across queues), streamed on one strictly-ordered
    queue in consumption order.
"""

import math
import os
import sys

sys.path.insert(0, "/opt/trn_rl_repo")

PHASES = os.environ.get("BASS_PHASES", "DEF")

import numpy as np

import concourse.bacc as bacc
import concourse.bass as bass
import concourse.tile as tile
from concourse import bass_utils, mybir

B, N, C = 2, 2048, 1024
H, HD = 16, 64
NCORES = 8
GH = 4  # head-group count (cores per batch)
NH = H // GH  # heads per core = 4
NG = 2 * NH  # LN groups per core (q heads + k heads) = 8
J = 3 * NH * HD  # qkv rows per core = 768
JA = J + NG  # + 8 mean columns = 776
NIN = N // 2  # rope positions restart here
NT = N // 128  # 16 row tiles
CCH = C // 128  # 8 contraction chunks
RB = NIN // 128  # rope table row blocks = 8
LOG_COND = math.log(0.5)
EPS = 1e-5
SCALE = HD ** -0.5  # 0.125

# Schraudolph fp16 exp: i16 = rint(x * SCH_S + SCH_C); bits as fp16.
SCH_S = float(2 ** 10) / math.log(2.0)
SCH_C = 15.0 * 2 ** 10 - 44.5

F32 = mybir.dt.float32
F16 = mybir.dt.float16
I16 = mybir.dt.int16
AF = mybir.ActivationFunctionType
AX = mybir.AxisListType
ALU = mybir.AluOpType

PAIRS = [(0, 0), (0, 1), (1, 0), (1, 1)]  # (nqh, hp)


def _bcast(ap, n, axis_insert=1):
    """Insert a 0-stride dim of size n into an AP (free-dim broadcast)."""
    new = list(ap.ap)
    new.insert(axis_insert, [0, n])
    return bass.AP(tensor=ap.tensor, offset=ap.offset, ap=new)


def build_nc(with_qb: bool, with_kb: bool, shared_t: bool = False, with_pb: bool = True):
    nc = bacc.Bacc("TRN2", target_bir_lowering=False, debug=False)

    # all inputs partition-major so each is one or two large contiguous
    # DMAs (the DMA queues pay ~0.7-1.3us dispatch latency per transfer)
    xt = nc.dram_tensor("xt", [128, CCH, N], F16, kind="ExternalInput")
    wt = nc.dram_tensor("wt", [128, CCH, JA], F16, kind="ExternalInput")
    pwt = nc.dram_tensor("pwt", [128, 2, C], F16, kind="ExternalInput")
    ident = nc.dram_tensor("ident", [128, 128], F16, kind="ExternalInput")
    nkinds_q = 6 if with_qb else 4
    nkinds_k = 6 if with_kb else 4
    tq = nc.dram_tensor("tq", [128, RB, nkinds_q, 32], F16, kind="ExternalInput")
    tk = None
    if not shared_t:
        tk = nc.dram_tensor("tk", [128, RB, nkinds_k, 32], F16, kind="ExternalInput")
    pb = None
    if with_pb:
        pb = nc.dram_tensor("pb", [C], F32, kind="ExternalInput")
    out_p = nc.dram_tensor("out_p", [N, C], F16, kind="ExternalOutput")
    dbg = None
    if os.environ.get("BASS_DEBUG"):
        dbg = {
            "dbg_qT": nc.dram_tensor("dbg_qT", [128, 2, N], F16, kind="ExternalOutput"),
            "dbg_kT": nc.dram_tensor("dbg_kT", [128, 2, N], F16, kind="ExternalOutput"),
            "dbg_oT": nc.dram_tensor("dbg_oT", [128, 2, N], F16, kind="ExternalOutput"),
            "dbg_v": nc.dram_tensor(
                "dbg_v", [128, NT, NH, HD + 1], F16, kind="ExternalOutput"
            ),
        }

    def rng(n, ph):
        return range(n if ph in PHASES else 0)

    with tile.TileContext(nc) as tc:
        with (
            tc.tile_pool(name="persist", bufs=1) as persist,
            tc.tile_pool(name="qkvps", bufs=3, space="PSUM") as qkvps,
            tc.tile_pool(name="tpps", bufs=2, space="PSUM") as tpps,
            tc.tile_pool(name="dwork", bufs=8) as dwork,
            tc.tile_pool(name="epool", bufs=32) as epool,
            tc.tile_pool(name="nwork", bufs=8) as nwork,
            tc.tile_pool(name="fwork", bufs=8) as fwork,
        ):
            # ---- persistent SBUF tensors --------------------------------
            xT_sb = persist.tile([128, CCH, N], F16)
            wT_sb = persist.tile([128, CCH, JA], F16)
            pwT_sb = persist.tile([128, 2, C], F16)
            v_sb = persist.tile([128, NT, NH, HD + 1], F16)
            qT_sb = persist.tile([128, 2, N], F16)
            kT_sb = persist.tile([128, 2, N], F16)
            oT_sb = persist.tile([128, 2, N], F16)
            tq_sb = persist.tile([128, RB, nkinds_q, 32], F16)
            id_sb = persist.tile([128, 128], F16)
            tk_sb = None
            if not shared_t:
                tk_sb = persist.tile([128, RB, nkinds_k, 32], F16)
            pb_rep = None

            cst = persist.tile([128, 4], F32)
            nc.vector.memset(cst[:, 0:1], EPS)
            nc.vector.memset(cst[:, 1:2], 0.0)
            nc.vector.memset(cst[:, 2:3], LOG_COND)
            nc.const_aps.aps[(F32, EPS)] = cst[:, 0:1]
            nc.const_aps.aps[(F32, 0.0)] = cst[:, 1:2]
            nc.const_aps.aps[(F32, LOG_COND)] = cst[:, 2:3]
            # Pin the sqrt act-table at t=0 (under the DMA shadow) so the
            # per-tile LN Sqrt never stalls on a table load.
            nc.scalar.activation(out=cst[:, 3:4], in_=cst[:, 0:1], func=AF.Sqrt)

            # ---- input DMAs in strict consumption order -----------------
            # sync queue carries the critical stream: half the weights, the
            # leading x columns, the rest of the weights, then x column
            # blocks in tile order, proj weights last.
            # single strictly-ordered queue: the DMA device is FIFO across
            # queues, so the critical stream must not interleave.
            nc.sync.dma_start(out=xT_sb[:, :, 0:128], in_=xt[:, :, 0:128])
            nc.sync.dma_start(out=wT_sb[:, 0:4, :], in_=wt[:, 0:4, :])
            nc.sync.dma_start(out=wT_sb[:, 4:8, :], in_=wt[:, 4:8, :])
            nc.sync.dma_start(out=tq_sb, in_=tq[:, :, :, :])
            if not shared_t:
                nc.sync.dma_start(out=tk_sb, in_=tk[:, :, :, :])
            nc.sync.dma_start(out=xT_sb[:, :, 128:640], in_=xt[:, :, 128:640])
            nc.sync.dma_start(out=id_sb, in_=ident[:, :])
            for c0, c1 in ((640, 1152), (1152, 1664), (1664, 2048)):
                nc.sync.dma_start(
                    out=xT_sb[:, :, c0:c1], in_=xt[:, :, c0:c1]
                )
            nc.sync.dma_start(out=pwT_sb, in_=pwt[:, :, :])
            if with_pb:
                pb_rep = persist.tile([128, C], F32)
                pb_ap = pb[:]
                nc.gpsimd.dma_start(
                    out=pb_rep,
                    in_=bass.AP(
                        tensor=pb_ap.tensor,
                        offset=pb_ap.offset,
                        ap=[[0, 128]] + list(pb_ap.ap),
                    ),
                )
            nc.vector.memset(v_sb[:, :, :, HD : HD + 1], 1.0)

            # ---- phase D: qkv matmul + LN + rope (transposes pipelined) --
            pend_t = []

            def flush_transpose():
                ii, qkr_i = pend_t.pop(0)
                tp = tpps.tile([128, 4, 128], F16, tag="tp")
                for hp in range(2):
                    nc.tensor.transpose(
                        tp[:, hp, :], qkr_i[:, 2 * hp : 2 * hp + 2, :], id_sb
                    )
                    nc.tensor.transpose(
                        tp[:, 2 + hp, :],
                        qkr_i[:, NH + 2 * hp : NH + 2 * hp + 2, :],
                        id_sb,
                    )
                # alternate the PSUM->SBUF eviction between DVE and ACT
                if ii % 2 == 0:
                    nc.vector.tensor_copy(
                        out=qT_sb[:, 0:2, ii * 128 : (ii + 1) * 128],
                        in_=tp[:, 0:2, :],
                    )
                    nc.scalar.copy(
                        out=kT_sb[:, 0:2, ii * 128 : (ii + 1) * 128],
                        in_=tp[:, 2:4, :],
                    )
                else:
                    nc.scalar.copy(
                        out=qT_sb[:, 0:2, ii * 128 : (ii + 1) * 128],
                        in_=tp[:, 0:2, :],
                    )
                    nc.vector.tensor_copy(
                        out=kT_sb[:, 0:2, ii * 128 : (ii + 1) * 128],
                        in_=tp[:, 2:4, :],
                    )

            for i in rng(NT, "D"):
                qp = qkvps.tile([128, 512], F32, tag="qp")
                # padded to the av-accumulator slot size (tag shared with
                # the attention phase's AV tiles)
                vp = qkvps.tile([128, 264], F32, tag="vp", padded_shape=[128, 390])
                for cc in range(CCH):
                    nc.tensor.matmul(
                        qp,
                        lhsT=xT_sb[:, cc, i * 128 : (i + 1) * 128],
                        rhs=wT_sb[:, cc, 0:512],
                        start=(cc == 0),
                        stop=(cc == CCH - 1),
                    )
                    nc.tensor.matmul(
                        vp,
                        lhsT=xT_sb[:, cc, i * 128 : (i + 1) * 128],
                        rhs=wT_sb[:, cc, 512:776],
                        start=(cc == 0),
                        stop=(cc == CCH - 1),
                    )
                # LN reads qp straight from PSUM (qp slots have ~3 tiles of
                # slack); sumsq/64 via ACT Square with scale=1/8 folded in,
                # f16 squares so the DVE reduce runs at 2 elem/cycle.
                qp3 = qp.rearrange("p (g d) -> p g d", g=NG)
                sq = dwork.tile([128, NG, HD], F16, tag="sq")
                sums = dwork.tile([128, 4, NG], F32, tag="sums")
                nc.scalar.activation(out=sq, in_=qp3, func=AF.Square, scale=0.125)
                # v eviction right after the Square: emitting it early keeps
                # the vp slot rotation off the LN chain's critical path.
                nc.scalar.copy(
                    out=v_sb[:, i, :, 0:HD],
                    in_=vp[:, 0:256].rearrange("p (h d) -> p h d", h=NH),
                )
                nc.vector.tensor_reduce(
                    out=sums[:, 1, :], in_=sq, axis=AX.X, op=ALU.add
                )
                # var = ss/64 - mu^2 ; rstd = 1/sqrt(var + eps)
                nc.vector.tensor_copy(out=sums[:, 0, :], in_=vp[:, 256:264])
                nc.vector.tensor_mul(
                    out=sums[:, 2, :],
                    in0=sums[:, 0, :],
                    in1=sums[:, 0, :],
                )
                nc.vector.tensor_sub(
                    out=sums[:, 1, :], in0=sums[:, 1, :], in1=sums[:, 2, :]
                )
                nc.scalar.activation(
                    out=sums[:, 1, :], in_=sums[:, 1, :], func=AF.Sqrt, bias=EPS
                )
                nc.vector.reciprocal(out=sums[:, 1, :], in_=sums[:, 1, :])
                # nb = (-mu) * rstd
                nc.vector.tensor_mul(
                    out=sums[:, 2, :], in0=sums[:, 0, :], in1=sums[:, 1, :]
                )
                qk_sb = dwork.tile([128, NG, HD], F16, tag="qk")
                for g in range(NG):
                    if g < 4:
                        nc.scalar.activation(
                            out=qk_sb[:, g, :],
                            in_=qp3[:, g, :],
                            func=AF.Identity,
                            bias=sums[:, 2, g : g + 1],
                            scale=sums[:, 1, g : g + 1],
                        )
                    else:
                        nc.vector.tensor_scalar(
                            out=qk_sb[:, g, :],
                            in0=qp3[:, g, :],
                            scalar1=sums[:, 1, g : g + 1],
                            scalar2=sums[:, 2, g : g + 1],
                            op0=ALU.mult,
                            op1=ALU.add,
                        )
                # rope (tables carry the LN weights; broadcast over groups)
                qkr = dwork.tile([128, NG, HD], F16, tag="qkr")
                r = i % RB
                if shared_t:
                    groups = ((tq_sb, 0, NG, with_qb),)
                else:
                    groups = (
                        (tq_sb, 0, NH, with_qb),
                        (tk_sb, NH, NH, with_kb),
                    )
                t_full = dwork.tile([128, NG, 32], F16, tag="ropetmp")
                t_full2 = dwork.tile([128, NG, 32], F16, tag="ropetmp2")
                for tsb, base, gn, wb in groups:
                    a1 = qk_sb[:, base : base + gn, 0:32]
                    a2 = qk_sb[:, base : base + gn, 32:64]
                    o1 = qkr[:, base : base + gn, 0:32]
                    o2 = qkr[:, base : base + gn, 32:64]
                    t = t_full[:, base : base + gn, :]
                    t2 = t_full2[:, base : base + gn, :]
                    tb = lambda k: _bcast(tsb[:, r, k, :], gn)
                    # o1 chain on DVE, o2 chain on Pool (both SBUF f16)
                    nc.vector.tensor_mul(out=t, in0=a1, in1=tb(0))
                    nc.vector.tensor_mul(out=o1, in0=a2, in1=tb(1))
                    nc.gpsimd.tensor_sub(out=o1, in0=t, in1=o1)
                    nc.vector.tensor_mul(out=t2, in0=a2, in1=tb(2))
                    nc.gpsimd.tensor_mul(out=o2, in0=a1, in1=tb(3))
                    nc.gpsimd.tensor_add(out=o2, in0=t2, in1=o2)
                    if wb:
                        nc.vector.tensor_add(out=o1, in0=o1, in1=tb(4))
                        nc.gpsimd.tensor_add(out=o2, in0=o2, in1=tb(5))
                pend_t.append((i, qkr))
                if len(pend_t) > 4:
                    flush_transpose()

            # Switch the act table to the exp set now, overlapped with the
            # rope/transpose tail (the load runs after tile 15's Sqrt).
            nc.scalar.activation(out=cst[:, 3:4], in_=cst[:, 1:2], func=AF.Exp)
            if "E" not in PHASES:
                while pend_t:
                    flush_transpose()

            # ---- phase E: attention (scoresT -> exp -> AV^T) + proj -----
            ocnt = [0]

            def proj_tile(i):
                # y[i*128:(i+1)*128, :] = o^T pw (+pb), streamed as 2 halves
                for oc in range(2):
                    op = tpps.tile([128, 512], F32, tag="tp")
                    for cc in range(2):
                        nc.tensor.matmul(
                            op,
                            lhsT=oT_sb[:, cc, i * 128 : (i + 1) * 128],
                            rhs=pwT_sb[:, cc, oc * 512 : (oc + 1) * 512],
                            start=(cc == 0),
                            stop=(cc == 1),
                        )
                    ot = fwork.tile([128, 512], F16, tag="ot")
                    if with_pb:
                        # DVE both halves: Pool has no PSUM read port
                        nc.vector.tensor_add(
                            out=ot, in0=op, in1=pb_rep[:, oc * 512 : (oc + 1) * 512]
                        )
                    else:
                        if oc == 0:
                            nc.vector.tensor_copy(out=ot, in_=op)
                        else:
                            nc.scalar.copy(out=ot, in_=op)
                    q = (nc.scalar, nc.sync)[ocnt[0] % 2]
                    ocnt[0] += 1
                    q.dma_start(
                        out=out_p[i * 128 : (i + 1) * 128, oc * 512 : (oc + 1) * 512],
                        in_=ot,
                    )

            pend_norm = []

            def norm_qt(o32_, row_, qt_, nqh_, hp_):
                # o = av[:, 0:64] / av[:, 64] per head, from the
                # SBUF-evacuated accumulator; the multiply runs on Pool
                # (SBUF-only engine), then PE-transpose to (d, q).
                av2 = o32_[:, row_, :].rearrange("p (z t) -> p z t", z=2)
                rcp = nwork.tile([128, 2], F32, tag="rcp")
                o16 = nwork.tile([128, 128], F16, tag="o16")
                nc.vector.reciprocal(
                    out=rcp.rearrange("p (z o) -> p z o", z=2),
                    in_=av2[:, :, 64:65],
                )
                nc.gpsimd.tensor_mul(
                    out=o16.rearrange("p (z d) -> p z d", z=2),
                    in0=av2[:, :, 0:64],
                    in1=_bcast(rcp, 64, axis_insert=2),
                )
                tp2 = tpps.tile([128, 128], F16, tag="tp")
                nc.tensor.transpose(tp2, o16, id_sb)
                col = nqh_ * 1024 + qt_ * 128
                nc.vector.tensor_copy(out=oT_sb[:, hp_, col : col + 128], in_=tp2)

            for pi in rng(4, "E"):
                nqh, hp = PAIRS[pi]
                avt = [
                    qkvps.tile([128, 3, 130], F32, tag="vp", name=f"av{pi}_{j}")
                    for j in range(3)
                ]
                pend_av = []

                def av_flush(qts=range(8), pop=True):
                    kc_, e0_, e1_ = pend_av[0]
                    for qt in qts:
                        a = avt[qt // 3]
                        row = qt % 3
                        for z in range(2):
                            e = (e0_, e1_)[z]
                            # start=True zeroes the WHOLE 2KB psum bank, so
                            # only the first region-matmul of each bank may
                            # carry it; the others accumulate onto zeros.
                            first_of_bank = qt % 3 == 0 and z == 0
                            nc.tensor.matmul(
                                a[:, row, z * 65 : (z + 1) * 65],
                                lhsT=e[:, qt * 128 : (qt + 1) * 128],
                                rhs=v_sb[:, kc_, 2 * hp + z, :],
                                start=(kc_ == 0 and first_of_bank),
                                stop=(kc_ == NT - 1),
                            )
                    if pop:
                        pend_av.pop(0)

                # the SBUF evacuation at pair end decouples norms from the
                # av PSUM slots, so a short AV lag suffices
                lag = 2 if pi == 3 else 4
                for kc in range(NT):
                    bias = 0.0 if ((kc < 8) == (nqh == 0)) else LOG_COND
                    e0 = epool.tile([128, 1024], F16, tag="E", name="e0")
                    e1 = epool.tile([128, 1024], F16, tag="E", name="e1")
                    # the 4th score tile borrows the (mostly idle) tp slots,
                    # widening the sp recycle loop from 3 to 5 buffers —
                    # except where proj competes for them.
                    tp_ok = not (pi == 2 and kc >= 8) and not (pi == 3 and kc >= 13)
                    for z in range(2):
                        e = (e0, e1)[z]
                        for half in range(2):
                            tag = "tp" if (z == 1 and half == 1 and tp_ok) else "qp"
                            sp = qkvps.tile(
                                [128, 512], F32, tag=tag, name="sp"
                            ) if tag == "qp" else tpps.tile(
                                [128, 512], F32, tag="tp", name="sp"
                            )
                            q0 = nqh * 1024 + half * 512
                            nc.tensor.matmul(
                                sp,
                                lhsT=kT_sb[
                                    z * 64 : (z + 1) * 64,
                                    hp,
                                    kc * 128 : (kc + 1) * 128,
                                ],
                                rhs=qT_sb[z * 64 : (z + 1) * 64, hp, q0 : q0 + 512],
                                start=True,
                                stop=True,
                            )
                            # exp engine split: ACT (native exp) takes both
                            # first halves plus z1's second half every 4th
                            # kc (9:7 ACT:DVE); DVE takes the rest via the
                            # Schraudolph bit-trick.
                            on_act = half == 0 or (z == 1 and kc % 4 == 3)
                            if on_act:
                                nc.scalar.activation(
                                    out=e[:, half * 512 : (half + 1) * 512],
                                    in_=sp,
                                    func=AF.Exp,
                                    bias=bias,
                                    scale=SCALE,
                                )
                            else:
                                nc.vector.tensor_scalar(
                                    out=e[:, half * 512 : (half + 1) * 512].bitcast(I16),
                                    in0=sp,
                                    scalar1=SCALE * SCH_S,
                                    scalar2=bias * SCH_S + SCH_C,
                                    op0=ALU.mult,
                                    op1=ALU.add,
                                )
                    pend_av.append((kc, e0, e1))
                    if len(pend_av) > lag:
                        av_flush()
                    # leftover work from the previous phase/pair is emitted
                    # AFTER this kc's score/exp chain so the aux-engine
                    # queues never delay the sp-slot recycle.
                    if pi == 0 and kc in (1, 3, 5, 7) and pend_t:
                        flush_transpose()
                    if pend_norm and kc % 2 == 1:
                        pend_norm.pop(0)()
                    if pi == 2 and kc >= 8 and "F" in PHASES:
                        proj_tile(kc - 8)
                # drain, then evacuate the closed accumulators to SBUF in
                # three wide copies: the PSUM slots free immediately and the
                # norm pipeline becomes SBUF-only (spreadable anywhere).
                if pi < 3:
                    while pend_av:
                        av_flush()
                    o32s = []
                    for j in range(3):
                        o32 = nwork.tile(
                            [128, 3, 130], F32, tag="o32", name=f"o32_{pi}_{j}"
                        )
                        if j == 1:
                            nc.scalar.copy(out=o32, in_=avt[j])
                        else:
                            nc.vector.tensor_copy(out=o32, in_=avt[j])
                        o32s.append(o32)
                    pend_norm = [
                        (lambda o=o32s[qt // 3], r=qt % 3, q=qt, n=nqh, h=hp:
                         norm_qt(o, r, q, n, h))
                        for qt in range(8)
                    ]
                else:
                    while len(pend_av) > 1:
                        av_flush()
                    done_qt = 0
                    for gi, qte in enumerate((3, 6, 8)):
                        av_flush(qts=range(done_qt, qte), pop=(qte == 8))
                        o32 = nwork.tile([128, 3, 130], F32, tag="o32")
                        if gi == 1:
                            nc.scalar.copy(out=o32, in_=avt[gi])
                        else:
                            nc.vector.tensor_copy(out=o32, in_=avt[gi])
                        for qt in range(done_qt, qte):
                            norm_qt(o32, qt % 3, qt, nqh, hp)
                            if "F" in PHASES:
                                proj_tile(8 + qt)
                        done_qt = qte
            while pend_norm:
                pend_norm.pop(0)()
            if dbg is not None:
                nc.sync.dma_start(out=dbg["dbg_qT"][:, :, :], in_=qT_sb)
                nc.sync.dma_start(out=dbg["dbg_kT"][:, :, :], in_=kT_sb)
                nc.sync.dma_start(out=dbg["dbg_oT"][:, :, :], in_=oT_sb)
                nc.sync.dma_start(out=dbg["dbg_v"][:, :, :, :], in_=v_sb)

    nc.compile()
    return nc


def _rope_tables(n_w, n_b, with_b):
    inv = 1.0 / (10000.0 ** (np.arange(0, HD, 2, dtype=np.float64) / HD))
    ang = np.arange(NIN, dtype=np.float64)[:, None] * inv[None, :]  # (NIN, 32)
    cos_h = np.cos(ang)
    sin_h = np.sin(ang)
    w1, w2 = n_w[:32].astype(np.float64), n_w[32:].astype(np.float64)
    b1, b2 = n_b[:32].astype(np.float64), n_b[32:].astype(np.float64)
    kinds = [w1 * cos_h, w2 * sin_h, w2 * cos_h, w1 * sin_h]
    if with_b:
        kinds += [b1 * cos_h - b2 * sin_h, b2 * cos_h + b1 * sin_h]
    t = np.stack(kinds, axis=1)  # (NIN, k, 32)
    k = t.shape[1]
    return np.ascontiguousarray(
        t.astype(np.float16).reshape(RB, 128, k, 32).transpose(1, 0, 2, 3)
    )


_NC_CACHE = {}


def kernel(x, qkv_w, qn_w, qn_b, kn_w, kn_b, proj_w, proj_b):
    x = np.asarray(x, np.float32)
    qkv_w = np.asarray(qkv_w, np.float32)
    proj_w = np.asarray(proj_w, np.float32)
    proj_b = np.asarray(proj_b, np.float32)
    qn_w = np.asarray(qn_w, np.float32)
    qn_b = np.asarray(qn_b, np.float32)
    kn_w = np.asarray(kn_w, np.float32)
    kn_b = np.asarray(kn_b, np.float32)

    with_qb = bool(np.any(qn_b != 0))
    with_kb = bool(np.any(kn_b != 0))
    shared_t = (
        with_qb == with_kb
        and np.array_equal(qn_w, kn_w)
        and np.array_equal(qn_b, kn_b)
    )
    with_pb = bool(np.any(proj_b != 0))
    key = (with_qb, with_kb, shared_t, with_pb)
    if key not in _NC_CACHE:
        _NC_CACHE[key] = build_nc(with_qb, with_kb, shared_t, with_pb)
    nc = _NC_CACHE[key]

    tq = _rope_tables(qn_w, qn_b, with_qb)
    tk = None if shared_t else _rope_tables(kn_w, kn_b, with_kb)

    ident = np.eye(128, dtype=np.float16)

    in_maps = []
    for core in range(NCORES):
        b, g = core // GH, core % GH
        rows = slice(g * NH * HD, (g + 1) * NH * HD)
        # column layout: [q(256) k(256)] then [v(256) mq(4) mk(4)]
        wq = qkv_w[rows]
        wk = qkv_w[C:][rows]
        wv = qkv_w[2 * C :][rows]
        # negated so the matmul mean columns land as -mu (saves a negate op)
        means = -np.stack(
            [wq[h * HD : (h + 1) * HD].mean(0) for h in range(NH)]
            + [wk[h * HD : (h + 1) * HD].mean(0) for h in range(NH)]
        )  # (8, C)
        w_aug = np.concatenate([wq, wk, wv, means], 0)  # (776, C)
        wt16 = np.ascontiguousarray(
            w_aug.T.reshape(CCH, 128, JA).transpose(1, 0, 2).astype(np.float16)
        )
        xt16 = np.ascontiguousarray(
            x[b].T.reshape(CCH, 128, N).transpose(1, 0, 2).astype(np.float16)
        )
        pwt16 = np.ascontiguousarray(
            proj_w[:, rows].T.reshape(2, 128, C).transpose(1, 0, 2).astype(np.float16)
        )
        im = {
            "xt": xt16,
            "wt": wt16,
            "pwt": pwt16,
            "ident": ident,
            "tq": tq,
        }
        if with_pb:
            im["pb"] = proj_b if g == 0 else np.zeros_like(proj_b)
        if tk is not None:
            im["tk"] = tk
        in_maps.append(im)

    res = bass_utils.run_bass_kernel_spmd(nc, in_maps, core_ids=list(range(NCORES)))
    parts = [r["out_p"].astype(np.float32) for r in res.results]
    out = np.stack(
        [np.sum(parts[b * GH : (b + 1) * GH], axis=0) for b in range(B)]
    )
    return out.astype(np.float32)


if __name__ == "__main__":
    rng = np.random.default_rng(0)
    ins = {
        "x": rng.standard_normal((B, N, C), np.float32),
        "qkv_w": (rng.standard_normal((3 * C, C), np.float32) / math.sqrt(C)).astype(
            np.float32
        ),
        "qn_w": np.ones(HD, np.float32),
        "qn_b": np.zeros(HD, np.float32),
        "kn_w": np.ones(HD, np.float32),
        "kn_b": np.zeros(HD, np.float32),
        "proj_w": (rng.standard_normal((C, C), np.float32) / math.sqrt(C)).astype(
            np.float32
        ),
        "proj_b": np.zeros(C, np.float32),
    }
    o = kernel(**ins)
    print(o.shape, o.dtype)
